# revision 5
# baseline (speedup 1.0000x reference)
"""Masked max-pool over span axis (MaxSpanRepr) on 8 Trainium2 cores.

Computation: out[b, l, d] = max_s( mask[b, s] ? spans[b, l, s, d] : -1e10 )
  spans          [2048, 13, 4, 1024] f32
  attention_mask [2048, 4] int32
  out            [2048, 13, 1024] f32

Strategy: data-parallel over batch, 256 examples per core. Per core the
spans shard is viewed as [3328 rows x 4096] (row = (b,l), contiguous
S*D block). Tiles of 256 rows (128 partitions x 2 rows) stream through
SBUF; the masked max is a chain of one tensor_scalar add plus three
scalar_tensor_tensor (add,max) ops per row-half, using a per-partition
bias column (-1e10 for masked s, 0 otherwise) precomputed on host from
the 8 KB mask.  The additive bias is exact in f32: |x| < 512 means
x + (-1e10) rounds to exactly -1e10, matching the reference's where().
"""

import numpy as np

import concourse.bass as bass
import concourse.mybir as mybir
from concourse.bass_utils import run_bass_kernel_spmd
from concourse.tile import TileContext

# The walrus build in this container supports a single sync-wait slot per
# instruction ("Too many sync wait commands" in setupSyncWait otherwise),
# while Tile freely attaches one wait per semaphore lane. Post-pass: for any
# instruction carrying N>1 waits, hoist N-1 of them onto NoOp instructions
# inserted just before it on the same engine (engines execute in order, so
# all waits still complete before the instruction runs).
def _split_multi_wait_instructions(nc):
    ctr = 0
    for fn in nc.m.functions:
        for blk in fn.blocks:
            insts = blk.instructions
            out = []
            changed = False
            for inst in insts:
                si = inst.sync_info
                waits = list(si.on_wait) if si is not None else []
                if len(waits) > 1:
                    changed = True
                    for w in waits[:-1]:
                        ctr += 1
                        nop = mybir.InstNoOp(
                            name=f"I-waitsplit-{ctr}", ins=[], outs=[])
                        nop.engine = inst.engine
                        nsi = mybir.SyncInfo(on_update=[], on_wait=[w])
                        nop.sync_info = nsi
                        out.append(nop)
                    si.on_wait = [waits[-1]]
                out.append(inst)
            if changed:
                blk.instructions = out

B, L, S, D = 2048, 13, 4, 1024
N_CORES = 8
B_SH = B // N_CORES              # 256 examples per core
ROWS = B_SH * L                  # 3328 (b,l) rows per core
RPT = 256                        # rows per tile: 128 partitions x 2 rows
N_TILES = ROWS // RPT            # 13
NEG_FILL = np.float32(-1e10)

_NC_CACHE = {}


def _build_nc():
    if "nc" in _NC_CACHE:
        return _NC_CACHE["nc"]
    nc = bass.Bass()
    spans = nc.dram_tensor("spans", [ROWS, S * D], mybir.dt.float32,
                           kind="ExternalInput")
    bias = nc.dram_tensor("bias", [128, N_TILES * 2 * S], mybir.dt.float32,
                          kind="ExternalInput")
    out = nc.dram_tensor("out", [ROWS, D], mybir.dt.float32,
                         kind="ExternalOutput")

    with TileContext(nc) as tc:
        with (
            tc.tile_pool(name="biasp", bufs=1) as bias_pool,
            tc.tile_pool(name="inp", bufs=4) as in_pool,
            tc.tile_pool(name="outp", bufs=4) as out_pool,
        ):
            bias_t = bias_pool.tile([128, N_TILES * 2 * S], mybir.dt.float32)
            nc.sync.dma_start(out=bias_t[:], in_=bias[:])
            for t in range(N_TILES):
                tin = in_pool.tile([128, 2 * S * D], mybir.dt.float32)
                nc.sync.dma_start(
                    out=tin[:],
                    in_=spans[t * RPT:(t + 1) * RPT, :].rearrange(
                        "(p two) d -> p (two d)", two=2),
                )
                tout = out_pool.tile([128, 2 * D], mybir.dt.float32)
                for h in range(2):
                    acc = tout[:, h * D:(h + 1) * D]
                    for s in range(S):
                        x = tin[:, h * S * D + s * D: h * S * D + (s + 1) * D]
                        c = t * 2 * S + h * S + s
                        bcol = bias_t[:, c:c + 1]
                        if s == 0:
                            nc.vector.tensor_scalar_add(
                                out=acc, in0=x, scalar1=bcol)
                        else:
                            nc.vector.scalar_tensor_tensor(
                                out=acc, in0=x, scalar=bcol, in1=acc,
                                op0=mybir.AluOpType.add,
                                op1=mybir.AluOpType.max)
                nc.sync.dma_start(
                    out=out[t * RPT:(t + 1) * RPT, :].rearrange(
                        "(p two) d -> p (two d)", two=2),
                    in_=tout[:],
                )
    _split_multi_wait_instructions(nc)
    _NC_CACHE["nc"] = nc
    return nc


def _make_in_maps(spans, attention_mask):
    spans = np.ascontiguousarray(np.asarray(spans, dtype=np.float32))
    mask = np.asarray(attention_mask)
    assert spans.shape == (B, L, S, D), spans.shape
    assert mask.shape == (B, S), mask.shape

    bias = np.where(mask == 0, NEG_FILL, np.float32(0.0)).astype(np.float32)
    bias_rows = np.repeat(bias, L, axis=0)               # [B*L, S]
    spans_flat = spans.reshape(B * L, S * D)

    in_maps = []
    for i in range(N_CORES):
        br = bias_rows[i * ROWS:(i + 1) * ROWS].reshape(N_TILES, 128, 2, S)
        bias_sb = np.ascontiguousarray(
            br.transpose(1, 0, 2, 3)).reshape(128, N_TILES * 2 * S)
        in_maps.append({
            "spans": spans_flat[i * ROWS:(i + 1) * ROWS],
            "bias": bias_sb,
        })
    return in_maps


def run(spans, attention_mask, **spmd_kwargs):
    """Run the device kernel; returns (full_output, BassKernelResults)."""
    nc = _build_nc()
    in_maps = _make_in_maps(spans, attention_mask)
    res = run_bass_kernel_spmd(nc, in_maps, core_ids=list(range(N_CORES)),
                               **spmd_kwargs)
    outs = [r["out"] for r in res.results]
    full = np.concatenate(outs, axis=0).reshape(B, L, D)
    return full, res


def kernel(spans, attention_mask):
    full, _ = run(spans, attention_mask)
    return full
